# Initial kernel scaffold
#
"""Trainium2 Bass kernel for the LayerNorm + tensor-train contraction net.

Math (per sample b):
    xn   = LayerNorm(x[b])                          # [D, L], stats over (D,L)
    t1   = xn[:,0] @ layer0                         # [R]
    t2_s = sum_{r,d} t1_r * xn[d,1] * core1[r,d,s]  # [S]
    t3_u = sum_{s,e} t2_s * xn[e,2] * core2[s,e,u]  # [U]
    out  = t3 @ last                                # [O]

Mapping (per core, pure batch data-parallel over 8 cores):
  - b-tiles of 128 samples live on SBUF partitions.
  - LN stats via bn_stats/bn_aggr in natural [b, (d l)] layout.
  - PE transposes produce xn_T[d, b] per l; the LN affine (w,b per (d,l))
    is folded into the PSUM->SBUF copy as per-partition scale/bias.
  - TT step k: Q[b,(s,r)] = sum_d xnk_T[d,b] * Cperm[d,(s,r)] on TensorE
    (xn_T is the stationary operand -> each output row b is that sample's
    own matvec), then t_next[b,s] = sum_r t_prev[b,r] * Q[b,(s,r)] as a
    DVE broadcast-multiply + inner-axis reduce.
  - `last` is folded into core2 on the host: C2'[s,e,o] = core2 @ last.
"""

import os
import sys

import numpy as np

try:
    import concourse.bass as bass  # noqa: F401
except Exception:  # pragma: no cover - fresh-dir fallback
    for p in ("/opt/trn_rl_repo", "/root/.axon_site/_ro/trn_rl_repo"):
        if os.path.isdir(p) and p not in sys.path:
            sys.path.insert(0, p)

import concourse.bass as bass
import concourse.tile as tile
from concourse import mybir

B, D, L, R, O = 32768, 128, 3, 64, 64
S = 64
EPS = 1e-5
N_CORES = 8
BC = B // N_CORES          # samples per core
P = 128                    # partition tile (samples per b-tile)
KW = R * S                 # 4096 columns of the permuted TT cores
N_MM = 512                 # matmul free-dim per instruction
CHUNK = 1024               # q-chunk columns (= 2 matmuls, 2 PSUM banks)
NCHUNK = KW // CHUNK       # 4
SG = CHUNK // R            # s-groups per chunk (16)

F32 = mybir.dt.float32
F32R = mybir.dt.float32r
BF16 = mybir.dt.bfloat16

# packed matmul-constant columns (dtype = MM_DT):
#   layer0 | c1p(s,r) | c2p(o,s) | c1p(r,s) | c2p(s,o)
CO_L0 = 0
CO_C1 = CO_L0 + R          # 64
CO_C2 = CO_C1 + KW         # 4160
CO_C1R = CO_C2 + KW        # 8256
CO_C2R = CO_C1R + KW       # 12352
CW = CO_C2R + KW           # 16448
# packed fp32 scalar/identity columns: ln_w | ln_b | eps | identity
C2_LNW = 0
C2_LNB = C2_LNW + L
C2_EPS = C2_LNB + L
C2_ID = C2_EPS + 1
CW2 = C2_ID + P

# Matmul input dtype for the TT matmuls: float32r runs at bf16 speed for
# free-dim >= 256 while keeping fp32 operand bits (reduced internal
# precision); bf16 is the fallback.
MM_DT_NAME = os.environ.get("KERNEL_MM_DT", "f32r")
MM_DT = {"f32": F32, "f32r": F32R, "bf16": BF16}[MM_DT_NAME]

# Of every 8 b-tiles, how many run the per-sample contraction on the Scalar
# engine (64 per-r activation ops + one strided DVE reduce) instead of the
# DVE broadcast-multiply path. Balances the DVE (bottleneck) against the
# mostly-idle ACT engine.
ACT_FRAC8 = int(os.environ.get("KERNEL_ACT_FRAC8", "0"))

# Benchmarking aid: wrap the whole tile loop in an on-device For_i hardware
# loop running REPS extra times. Wall-clock deltas across REPS isolate pure
# on-device compute from the (large, constant) transfer/dispatch cost.
REPS = int(os.environ.get("KERNEL_REPS", "0"))


def _legalize_sync(nc, max_waits=1, max_updates=1):
    """Split multi-wait/multi-update sync_info into standalone EventSemaphore
    instructions.

    The walrus build in this environment encodes at most one sync wait (and
    one update) per 64B TPB instruction and refuses to split ("Too many sync
    wait commands"), while Tile emits instructions waiting on several
    semaphores. Engine queues execute in order, so hoisting the overflow
    waits into EventSemaphore instructions immediately before (and overflow
    updates immediately after) the instruction preserves semantics.
    """
    import json

    bir = json.loads(mybir.module_to_json_bytes(nc.m))
    uid = [0]
    for fn in bir["functions"]:
        for blk in fn["blocks"]:
            new_insts = []
            for inst in blk["instructions"]:
                sync = inst.get("sync_info")
                if not sync:
                    new_insts.append(inst)
                    continue
                waits = sync.get("on_wait") or []
                ups = sync.get("on_update") or []
                eng = inst.get("engine")
                for w in waits[max_waits:]:
                    uid[0] += 1
                    new_insts.append({
                        "debug": inst.get("debug", 0),
                        "engine": eng,
                        "ins": [],
                        "name": f"legw-{uid[0]}",
                        "opcode": "EventSemaphore",
                        "outs": [],
                        "sync_info": {"on_update": [], "on_wait": [w]},
                    })
                sync["on_wait"] = waits[:max_waits]
                new_insts.append(inst)
                for u in ups[max_updates:]:
                    uid[0] += 1
                    new_insts.append({
                        "debug": inst.get("debug", 0),
                        "engine": eng,
                        "ins": [],
                        "name": f"legu-{uid[0]}",
                        "opcode": "EventSemaphore",
                        "outs": [],
                        "sync_info": {"on_update": [u], "on_wait": []},
                    })
                sync["on_update"] = ups[:max_updates]
            blk["instructions"] = new_insts
    nc.m = mybir.module_from_json_bytes(json.dumps(bir).encode())
    return nc


def _build_program(n_tiles: int, legalize: bool = True):
    """Emit the single-core Bass/Tile program processing n_tiles*128 samples."""
    from contextlib import ExitStack

    bc = n_tiles * P
    nc = bass.Bass()
    xs = nc.declare_dram_parameter("xs", [bc, D * L], F32, isOutput=False)
    cst = nc.declare_dram_parameter("cst", [P, CW], MM_DT, isOutput=False)
    cst2 = nc.declare_dram_parameter("cst2", [P, CW2], F32, isOutput=False)
    out = nc.declare_dram_parameter("out", [bc, O], F32, isOutput=True)

    Ident = mybir.ActivationFunctionType.Identity
    Sqrt = mybir.ActivationFunctionType.Sqrt
    AX = mybir.AxisListType.X
    MUL = mybir.AluOpType.mult
    ADD = mybir.AluOpType.add

    with tile.TileContext(nc) as tc, ExitStack() as ctx:
        consts = ctx.enter_context(tc.tile_pool(name="consts", bufs=1))
        xpool = ctx.enter_context(tc.tile_pool(name="xp", bufs=3))
        ypool = ctx.enter_context(tc.tile_pool(name="yp", bufs=2))
        xnt_pool = ctx.enter_context(tc.tile_pool(name="xnt", bufs=2))
        stat_pool = ctx.enter_context(tc.tile_pool(name="stats", bufs=3))
        tvec_pool = ctx.enter_context(tc.tile_pool(name="tvec", bufs=2))
        ppool = ctx.enter_context(tc.tile_pool(name="pp", bufs=3))
        qpsum = ctx.enter_context(tc.tile_pool(name="qp", bufs=2, space="PSUM"))
        trpsum = ctx.enter_context(tc.tile_pool(name="trp", bufs=2, space="PSUM"))
        t1psum = ctx.enter_context(tc.tile_pool(name="t1p", bufs=2, space="PSUM"))

        # --- constants: one packed tile per dtype ---
        cst_sb = consts.tile([P, CW], MM_DT)
        nc.sync.dma_start(out=cst_sb[:], in_=cst[:, :])
        l0_sb = cst_sb[:, CO_L0:CO_L0 + R]
        c1p_sb = cst_sb[:, CO_C1:CO_C1 + KW]
        c2p_sb = cst_sb[:, CO_C2:CO_C2 + KW]
        c1r_sb = cst_sb[:, CO_C1R:CO_C1R + KW]
        c2r_sb = cst_sb[:, CO_C2R:CO_C2R + KW]
        cst2_sb = consts.tile([P, CW2], F32)
        nc.sync.dma_start(out=cst2_sb[:], in_=cst2[:, :])
        lnw_sb = cst2_sb[:, C2_LNW:C2_LNW + L]
        lnb_sb = cst2_sb[:, C2_LNB:C2_LNB + L]
        eps_sb = cst2_sb[:, C2_EPS:C2_EPS + 1]
        ident = cst2_sb[:, C2_ID:C2_ID + P]

        def tt_step(xnt, cperm_sb, tprev_sb, dst_sb):
            """dst[b, s] = sum_r tprev[b, r] * (xnt.T @ cperm)[b, (s, r)].

            DVE path: cperm is (s-major, r-minor); per chunk one broadcast
            tensor_tensor multiply + one inner-axis reduce.
            """
            for c in range(NCHUNK):
                q = qpsum.tile([P, CHUNK], F32, tag="q")
                for n in range(CHUNK // N_MM):
                    col = c * CHUNK + n * N_MM
                    nc.tensor.matmul(
                        q[:, n * N_MM:(n + 1) * N_MM],
                        xnt[:, :],
                        cperm_sb[:, col:col + N_MM],
                        start=True,
                        stop=True,
                    )
                prod = ppool.tile([P, SG, R], F32, tag="prod")
                q3 = q[:, :].rearrange("p (s r) -> p s r", r=R)
                t_bc = tprev_sb[:, :].unsqueeze(1).broadcast_to((P, SG, R))
                nc.vector.tensor_tensor(out=prod[:], in0=q3, in1=t_bc, op=MUL)
                nc.vector.tensor_reduce(
                    out=dst_sb[:, c * SG:(c + 1) * SG],
                    in_=prod[:],
                    axis=AX,
                    op=ADD,
                )

        def tt_step_act(xnt, cperm_sb, tprev_sb, dst_sb):
            """Same contraction, (r-major, s-minor) constants: the per-r
            multiplies run on the Scalar engine (per-partition scale straight
            from PSUM), then one strided inner-axis DVE reduce."""
            prodr = ppool.tile([P, R, S], F32, tag="prodr")
            for c in range(NCHUNK):
                q = qpsum.tile([P, CHUNK], F32, tag="q")
                for n in range(CHUNK // N_MM):
                    col = c * CHUNK + n * N_MM
                    nc.tensor.matmul(
                        q[:, n * N_MM:(n + 1) * N_MM],
                        xnt[:, :],
                        cperm_sb[:, col:col + N_MM],
                        start=True,
                        stop=True,
                    )
                for rl in range(CHUNK // S):
                    r = c * (CHUNK // S) + rl
                    nc.scalar.mul(
                        prodr[:, r, :],
                        q[:, rl * S:(rl + 1) * S],
                        tprev_sb[:, r:r + 1],
                    )
            nc.vector.tensor_reduce(
                out=dst_sb[:, :],
                in_=prodr[:, :, :].transpose([0, 2, 1]),
                axis=AX,
                op=ADD,
            )

        def tile_body(i):
            row = i * P
            x_t = xpool.tile([P, D * L], F32, tag="x")
            nc.sync.dma_start(out=x_t[:], in_=xs[row:row + P, :])

            # LayerNorm stats over the full (d, l) row
            st = stat_pool.tile([P, 6], F32, tag="bn")
            nc.vector.bn_stats(out=st[:], in_=x_t[:])
            mv = stat_pool.tile([P, 2], F32, tag="mv")
            nc.vector.bn_aggr(out=mv[:], in_=st[:])
            # rstd = 1/sqrt(var + eps)
            rstd = stat_pool.tile([P, 1], F32, tag="rstd")
            nc.scalar.activation(out=rstd[:], in_=mv[:, 1:2], func=Sqrt,
                                 bias=eps_sb[:, :], scale=1.0)
            nc.vector.reciprocal(out=rstd[:], in_=rstd[:])
            # nmr = -mean * rstd
            nmr = stat_pool.tile([P, 1], F32, tag="nmr")
            nc.vector.scalar_tensor_tensor(
                out=nmr[:], in0=mv[:, 0:1], scalar=-1.0, in1=rstd[:],
                op0=MUL, op1=MUL)
            # y = (x - mean) * rstd
            y = ypool.tile([P, D * L], F32, tag="y")
            nc.scalar.activation(out=y[:], in_=x_t[:], func=Ident,
                                 bias=nmr[:, :], scale=rstd[:, :])

            # xn_T[d, b] per l, with the affine fold on the way out of PSUM
            y3 = y[:, :].rearrange("p (d l) -> p d l", l=L)
            xnt = []
            for l in range(L):
                tr = trpsum.tile([P, P], F32, tag="tr")
                nc.tensor.transpose(tr[:], y3[:, :, l], ident)
                xl = xnt_pool.tile([P, P], MM_DT, tag=f"xnt{l}")
                nc.scalar.activation(out=xl[:], in_=tr[:], func=Ident,
                                     bias=lnb_sb[:, l:l + 1],
                                     scale=lnw_sb[:, l:l + 1])
                xnt.append(xl)

            # t1[b, r] = sum_d xn0_T[d, b] * layer0[d, r]
            t1_ps = t1psum.tile([P, R], F32, tag="t1")
            nc.tensor.matmul(t1_ps[:], xnt[0][:, :], l0_sb[:, :],
                             start=True, stop=True)
            t1_sb = tvec_pool.tile([P, R], F32, tag="t1s")
            nc.scalar.copy(t1_sb[:], t1_ps[:])

            use_act = (i % 8) < ACT_FRAC8
            t2_sb = tvec_pool.tile([P, S], F32, tag="t2s")
            o_sb = tvec_pool.tile([P, O], F32, tag="os")
            if use_act:
                tt_step_act(xnt[1], c1r_sb, t1_sb, t2_sb)
                tt_step_act(xnt[2], c2r_sb, t2_sb, o_sb)
            else:
                tt_step(xnt[1], c1p_sb, t1_sb, t2_sb)
                tt_step(xnt[2], c2p_sb, t2_sb, o_sb)

            nc.sync.dma_start(out=out[row:row + P, :], in_=o_sb[:])

        if REPS:
            with tc.For_i(0, REPS, 1):
                for i in range(n_tiles):
                    tile_body(i)
        else:
            for i in range(n_tiles):
                tile_body(i)

    return _legalize_sync(nc) if legalize else nc


def _prep_consts(layer0, core1, core2, last, ln_w, ln_b):
    """Host-side constant massaging into packed per-dtype arrays."""
    layer0 = np.asarray(layer0, np.float32)
    core1 = np.asarray(core1, np.float32)
    core2 = np.asarray(core2, np.float32)
    last = np.asarray(last, np.float32)
    # c1p[d, s*R + r] = core1[r, d, s]
    c1p = core1.transpose(1, 2, 0).reshape(D, S * R)
    # fold `last` into core2: C2'[s, e, o] = sum_u core2[s,e,u] last[u,o]
    c2e = np.einsum("seu,uo->seo", core2, last).astype(np.float32)
    # c2p[e, o*S + s] = C2'[s, e, o]
    c2p = c2e.transpose(1, 2, 0).reshape(D, O * S)
    # (r-major, s-minor) variants for the ACT-path tiles
    c1r = core1.transpose(1, 0, 2).reshape(D, R * S)
    c2r = c2e.transpose(1, 0, 2).reshape(D, S * O)
    packed = np.concatenate([layer0, c1p, c2p, c1r, c2r], axis=1)
    assert packed.shape == (P, CW), packed.shape
    if MM_DT_NAME == "bf16":
        import ml_dtypes
        packed = packed.astype(ml_dtypes.bfloat16)
    eps = np.full((P, 1), EPS, np.float32)
    ident = np.eye(P, dtype=np.float32)
    packed2 = np.concatenate(
        [np.asarray(ln_w, np.float32), np.asarray(ln_b, np.float32),
         eps, ident], axis=1)
    assert packed2.shape == (P, CW2), packed2.shape
    return {"cst": np.ascontiguousarray(packed),
            "cst2": np.ascontiguousarray(packed2)}


_cached_nc = None
last_results = None  # BassKernelResults of the most recent run (for timing)


def kernel(x, layer0, core1, core2, last, ln_w, ln_b, trace=False,
           trace_kwargs=None):
    global _cached_nc, last_results
    from concourse.bass_utils import run_bass_kernel_spmd

    x = np.asarray(x, np.float32)
    consts = _prep_consts(layer0, core1, core2, last, ln_w, ln_b)

    if _cached_nc is None:
        _cached_nc = _build_program(BC // P)
    nc = _cached_nc

    xflat = np.ascontiguousarray(x.reshape(B, D * L))
    in_maps = []
    for ci in range(N_CORES):
        m = dict(consts)
        m["xs"] = xflat[ci * BC:(ci + 1) * BC]
        in_maps.append(m)

    kw = {}
    if trace:
        kw["trace"] = True
        kw.update(trace_kwargs or {})
    res = run_bass_kernel_spmd(nc, in_maps, list(range(N_CORES)), **kw)
    last_results = res
    full = np.concatenate(
        [np.asarray(res.results[ci]["out"]) for ci in range(N_CORES)], 0)
    return np.ascontiguousarray(full.astype(np.float32, copy=False))



# revision 1
# speedup vs baseline: 1.0374x; 1.0374x over previous
"""Trainium2 Bass kernel for the LayerNorm + tensor-train contraction net.

Math (per sample b):
    xn   = LayerNorm(x[b])                          # [D, L], stats over (D,L)
    t1   = xn[:,0] @ layer0                         # [R]
    t2_s = sum_{r,d} t1_r * xn[d,1] * core1[r,d,s]  # [S]
    t3_u = sum_{s,e} t2_s * xn[e,2] * core2[s,e,u]  # [U]
    out  = t3 @ last                                # [O]

Mapping (per core, pure batch data-parallel over 8 cores):
  - b-tiles of 128 samples live on SBUF partitions.
  - LN stats via bn_stats/bn_aggr in natural [b, (d l)] layout.
  - PE transposes produce xn_T[d, b] per l; the LN affine (w,b per (d,l))
    is folded into the PSUM->SBUF copy as per-partition scale/bias.
  - TT step k: Q[b,(s,r)] = sum_d xnk_T[d,b] * Cperm[d,(s,r)] on TensorE
    (xn_T is the stationary operand -> each output row b is that sample's
    own matvec), then t_next[b,s] = sum_r t_prev[b,r] * Q[b,(s,r)] as a
    DVE broadcast-multiply + inner-axis reduce.
  - `last` is folded into core2 on the host: C2'[s,e,o] = core2 @ last.
"""

import os
import sys

import numpy as np

try:
    import concourse.bass as bass  # noqa: F401
except Exception:  # pragma: no cover - fresh-dir fallback
    for p in ("/opt/trn_rl_repo", "/root/.axon_site/_ro/trn_rl_repo"):
        if os.path.isdir(p) and p not in sys.path:
            sys.path.insert(0, p)

import concourse.bass as bass
import concourse.tile as tile
from concourse import mybir

B, D, L, R, O = 32768, 128, 3, 64, 64
S = 64
EPS = 1e-5
N_CORES = 8
BC = B // N_CORES          # samples per core
P = 128                    # partition tile (samples per b-tile)
KW = R * S                 # 4096 columns of the permuted TT cores
N_MM = 512                 # matmul free-dim per instruction
CHUNK = 1024               # q-chunk columns (= 2 matmuls, 2 PSUM banks)
NCHUNK = KW // CHUNK       # 4
SG = CHUNK // R            # s-groups per chunk (16)

F32 = mybir.dt.float32
F32R = mybir.dt.float32r
BF16 = mybir.dt.bfloat16

# packed matmul-constant columns (dtype = MM_DT):
#   layer0 | c1p(s,r) | c2p(o,s) | c1p(r,s) | c2p(s,o)
CO_L0 = 0
CO_C1 = CO_L0 + R          # 64
CO_C2 = CO_C1 + KW         # 4160
CO_C1R = CO_C2 + KW        # 8256
CO_C2R = CO_C1R + KW       # 12352
CW = CO_C2R + KW           # 16448
# packed fp32 scalar/identity columns: ln_w | ln_b | eps | identity
C2_LNW = 0
C2_LNB = C2_LNW + L
C2_EPS = C2_LNB + L
C2_ID = C2_EPS + 1
CW2 = C2_ID + P

# Matmul input dtype for the TT matmuls: float32r runs at bf16 speed for
# free-dim >= 256 while keeping fp32 operand bits (reduced internal
# precision); bf16 is the fallback.
MM_DT_NAME = os.environ.get("KERNEL_MM_DT", "f32r")
MM_DT = {"f32": F32, "f32r": F32R, "bf16": BF16}[MM_DT_NAME]

# Of every 8 b-tiles, how many run the per-sample contraction on the Scalar
# engine (64 per-r activation ops + one strided DVE reduce) instead of the
# DVE broadcast-multiply path. Balances the DVE (bottleneck) against the
# mostly-idle ACT engine.
ACT_FRAC8 = int(os.environ.get("KERNEL_ACT_FRAC8", "0"))

# Benchmarking aid: wrap the whole tile loop in an on-device For_i hardware
# loop running REPS extra times. Wall-clock deltas across REPS isolate pure
# on-device compute from the (large, constant) transfer/dispatch cost.
REPS = int(os.environ.get("KERNEL_REPS", "0"))


def _legalize_sync(nc, max_waits=1, max_updates=1):
    """Split multi-wait/multi-update sync_info into standalone EventSemaphore
    instructions.

    The walrus build in this environment encodes at most one sync wait (and
    one update) per 64B TPB instruction and refuses to split ("Too many sync
    wait commands"), while Tile emits instructions waiting on several
    semaphores. Engine queues execute in order, so hoisting the overflow
    waits into EventSemaphore instructions immediately before (and overflow
    updates immediately after) the instruction preserves semantics.
    """
    import json

    bir = json.loads(mybir.module_to_json_bytes(nc.m))
    uid = [0]
    for fn in bir["functions"]:
        for blk in fn["blocks"]:
            new_insts = []
            for inst in blk["instructions"]:
                sync = inst.get("sync_info")
                if not sync:
                    new_insts.append(inst)
                    continue
                waits = sync.get("on_wait") or []
                ups = sync.get("on_update") or []
                eng = inst.get("engine")
                for w in waits[max_waits:]:
                    uid[0] += 1
                    new_insts.append({
                        "debug": inst.get("debug", 0),
                        "engine": eng,
                        "ins": [],
                        "name": f"legw-{uid[0]}",
                        "opcode": "EventSemaphore",
                        "outs": [],
                        "sync_info": {"on_update": [], "on_wait": [w]},
                    })
                sync["on_wait"] = waits[:max_waits]
                new_insts.append(inst)
                for u in ups[max_updates:]:
                    uid[0] += 1
                    new_insts.append({
                        "debug": inst.get("debug", 0),
                        "engine": eng,
                        "ins": [],
                        "name": f"legu-{uid[0]}",
                        "opcode": "EventSemaphore",
                        "outs": [],
                        "sync_info": {"on_update": [u], "on_wait": []},
                    })
                sync["on_update"] = ups[:max_updates]
            blk["instructions"] = new_insts
    nc.m = mybir.module_from_json_bytes(json.dumps(bir).encode())
    return nc


def _build_program(n_tiles: int, legalize: bool = True):
    """Emit the single-core Bass/Tile program processing n_tiles*128 samples."""
    from contextlib import ExitStack

    bc = n_tiles * P
    nc = bass.Bass()
    xs = nc.declare_dram_parameter("xs", [bc, D * L], F32, isOutput=False)
    cst = nc.declare_dram_parameter("cst", [P, CW], MM_DT, isOutput=False)
    cst2 = nc.declare_dram_parameter("cst2", [P, CW2], F32, isOutput=False)
    out = nc.declare_dram_parameter("out", [bc, O], F32, isOutput=True)

    Ident = mybir.ActivationFunctionType.Identity
    Sqrt = mybir.ActivationFunctionType.Sqrt
    AX = mybir.AxisListType.X
    MUL = mybir.AluOpType.mult
    ADD = mybir.AluOpType.add

    with tile.TileContext(nc) as tc, ExitStack() as ctx:
        consts = ctx.enter_context(tc.tile_pool(name="consts", bufs=1))
        xpool = ctx.enter_context(tc.tile_pool(name="xp", bufs=3))
        ypool = ctx.enter_context(tc.tile_pool(name="yp", bufs=2))
        xnt_pool = ctx.enter_context(tc.tile_pool(name="xnt", bufs=2))
        stat_pool = ctx.enter_context(tc.tile_pool(name="stats", bufs=3))
        tvec_pool = ctx.enter_context(tc.tile_pool(name="tvec", bufs=2))
        ppool = ctx.enter_context(tc.tile_pool(name="pp", bufs=3))
        qpsum = ctx.enter_context(tc.tile_pool(name="qp", bufs=2, space="PSUM"))
        trpsum = ctx.enter_context(tc.tile_pool(name="trp", bufs=2, space="PSUM"))
        t1psum = ctx.enter_context(tc.tile_pool(name="t1p", bufs=2, space="PSUM"))

        # --- constants: one packed tile per dtype ---
        cst_sb = consts.tile([P, CW], MM_DT)
        nc.sync.dma_start(out=cst_sb[:], in_=cst[:, :])
        l0_sb = cst_sb[:, CO_L0:CO_L0 + R]
        c1p_sb = cst_sb[:, CO_C1:CO_C1 + KW]
        c2p_sb = cst_sb[:, CO_C2:CO_C2 + KW]
        c1r_sb = cst_sb[:, CO_C1R:CO_C1R + KW]
        c2r_sb = cst_sb[:, CO_C2R:CO_C2R + KW]
        cst2_sb = consts.tile([P, CW2], F32)
        nc.sync.dma_start(out=cst2_sb[:], in_=cst2[:, :])
        lnw_sb = cst2_sb[:, C2_LNW:C2_LNW + L]
        lnb_sb = cst2_sb[:, C2_LNB:C2_LNB + L]
        eps_sb = cst2_sb[:, C2_EPS:C2_EPS + 1]
        ident = cst2_sb[:, C2_ID:C2_ID + P]

        def tt_step(xnt, cperm_sb, tprev_sb, dst_sb):
            """dst[b, s] = sum_r tprev[b, r] * (xnt.T @ cperm)[b, (s, r)].

            DVE path: cperm is (s-major, r-minor); per chunk one broadcast
            tensor_tensor multiply + one inner-axis reduce.
            """
            for c in range(NCHUNK):
                q = qpsum.tile([P, CHUNK], F32, tag="q")
                for n in range(CHUNK // N_MM):
                    col = c * CHUNK + n * N_MM
                    nc.tensor.matmul(
                        q[:, n * N_MM:(n + 1) * N_MM],
                        xnt[:, :],
                        cperm_sb[:, col:col + N_MM],
                        start=True,
                        stop=True,
                    )
                prod = ppool.tile([P, SG, R], F32, tag="prod")
                q3 = q[:, :].rearrange("p (s r) -> p s r", r=R)
                t_bc = tprev_sb[:, :].unsqueeze(1).broadcast_to((P, SG, R))
                nc.vector.tensor_tensor(out=prod[:], in0=q3, in1=t_bc, op=MUL)
                nc.vector.tensor_reduce(
                    out=dst_sb[:, c * SG:(c + 1) * SG],
                    in_=prod[:],
                    axis=AX,
                    op=ADD,
                )

        def tt_step_act(xnt, cperm_sb, tprev_sb, dst_sb):
            """Same contraction, (r-major, s-minor) constants: the per-r
            multiplies run on the Scalar engine (per-partition scale straight
            from PSUM), then one strided inner-axis DVE reduce."""
            prodr = ppool.tile([P, R, S], F32, tag="prodr")
            for c in range(NCHUNK):
                q = qpsum.tile([P, CHUNK], F32, tag="q")
                for n in range(CHUNK // N_MM):
                    col = c * CHUNK + n * N_MM
                    nc.tensor.matmul(
                        q[:, n * N_MM:(n + 1) * N_MM],
                        xnt[:, :],
                        cperm_sb[:, col:col + N_MM],
                        start=True,
                        stop=True,
                    )
                for rl in range(CHUNK // S):
                    r = c * (CHUNK // S) + rl
                    nc.scalar.mul(
                        prodr[:, r, :],
                        q[:, rl * S:(rl + 1) * S],
                        tprev_sb[:, r:r + 1],
                    )
            nc.vector.tensor_reduce(
                out=dst_sb[:, :],
                in_=prodr[:, :, :].transpose([0, 2, 1]),
                axis=AX,
                op=ADD,
            )

        def tile_body(i):
            row = i * P
            x_t = xpool.tile([P, D * L], F32, tag="x")
            nc.sync.dma_start(out=x_t[:], in_=xs[row:row + P, :])

            # LayerNorm stats over the full (d, l) row
            st = stat_pool.tile([P, 6], F32, tag="bn")
            nc.vector.bn_stats(out=st[:], in_=x_t[:])
            mv = stat_pool.tile([P, 2], F32, tag="mv")
            nc.vector.bn_aggr(out=mv[:], in_=st[:])
            # rstd = 1/sqrt(var + eps)
            rstd = stat_pool.tile([P, 1], F32, tag="rstd")
            nc.scalar.activation(out=rstd[:], in_=mv[:, 1:2], func=Sqrt,
                                 bias=eps_sb[:, :], scale=1.0)
            nc.vector.reciprocal(out=rstd[:], in_=rstd[:])
            # nmr = -mean * rstd
            nmr = stat_pool.tile([P, 1], F32, tag="nmr")
            nc.vector.scalar_tensor_tensor(
                out=nmr[:], in0=mv[:, 0:1], scalar=-1.0, in1=rstd[:],
                op0=MUL, op1=MUL)
            # y = (x - mean) * rstd
            y = ypool.tile([P, D * L], F32, tag="y")
            nc.scalar.activation(out=y[:], in_=x_t[:], func=Ident,
                                 bias=nmr[:, :], scale=rstd[:, :])

            # xn_T[d, b] per l, with the affine fold on the way out of PSUM
            y3 = y[:, :].rearrange("p (d l) -> p d l", l=L)
            xnt = []
            for l in range(L):
                tr = trpsum.tile([P, P], F32, tag="tr")
                nc.tensor.transpose(tr[:], y3[:, :, l], ident)
                xl = xnt_pool.tile([P, P], MM_DT, tag=f"xnt{l}")
                nc.scalar.activation(out=xl[:], in_=tr[:], func=Ident,
                                     bias=lnb_sb[:, l:l + 1],
                                     scale=lnw_sb[:, l:l + 1])
                xnt.append(xl)

            # t1[b, r] = sum_d xn0_T[d, b] * layer0[d, r]
            t1_ps = t1psum.tile([P, R], F32, tag="t1")
            nc.tensor.matmul(t1_ps[:], xnt[0][:, :], l0_sb[:, :],
                             start=True, stop=True)
            t1_sb = tvec_pool.tile([P, R], F32, tag="t1s")
            nc.scalar.copy(t1_sb[:], t1_ps[:])

            use_act = (i % 8) < ACT_FRAC8
            t2_sb = tvec_pool.tile([P, S], F32, tag="t2s")
            o_sb = tvec_pool.tile([P, O], F32, tag="os")
            if use_act:
                tt_step_act(xnt[1], c1r_sb, t1_sb, t2_sb)
                tt_step_act(xnt[2], c2r_sb, t2_sb, o_sb)
            else:
                tt_step(xnt[1], c1p_sb, t1_sb, t2_sb)
                tt_step(xnt[2], c2p_sb, t2_sb, o_sb)

            nc.sync.dma_start(out=out[row:row + P, :], in_=o_sb[:])

        if REPS:
            with tc.For_i(0, REPS, 1):
                for i in range(n_tiles):
                    tile_body(i)
        else:
            for i in range(n_tiles):
                tile_body(i)

    return _legalize_sync(nc) if legalize else nc


def _prep_consts(layer0, core1, core2, last, ln_w, ln_b):
    """Host-side constant massaging into packed per-dtype arrays."""
    layer0 = np.asarray(layer0, np.float32)
    core1 = np.asarray(core1, np.float32)
    core2 = np.asarray(core2, np.float32)
    last = np.asarray(last, np.float32)
    # c1p[d, s*R + r] = core1[r, d, s]
    c1p = core1.transpose(1, 2, 0).reshape(D, S * R)
    # fold `last` into core2: C2'[s, e, o] = sum_u core2[s,e,u] last[u,o]
    c2e = np.einsum("seu,uo->seo", core2, last).astype(np.float32)
    # c2p[e, o*S + s] = C2'[s, e, o]
    c2p = c2e.transpose(1, 2, 0).reshape(D, O * S)
    # (r-major, s-minor) variants for the ACT-path tiles
    c1r = core1.transpose(1, 0, 2).reshape(D, R * S)
    c2r = c2e.transpose(1, 0, 2).reshape(D, S * O)
    packed = np.concatenate([layer0, c1p, c2p, c1r, c2r], axis=1)
    assert packed.shape == (P, CW), packed.shape
    if MM_DT_NAME == "bf16":
        import ml_dtypes
        packed = packed.astype(ml_dtypes.bfloat16)
    eps = np.full((P, 1), EPS, np.float32)
    ident = np.eye(P, dtype=np.float32)
    packed2 = np.concatenate(
        [np.asarray(ln_w, np.float32), np.asarray(ln_b, np.float32),
         eps, ident], axis=1)
    assert packed2.shape == (P, CW2), packed2.shape
    return {"cst": np.ascontiguousarray(packed),
            "cst2": np.ascontiguousarray(packed2)}


_cached_nc = None
last_results = None  # BassKernelResults of the most recent run (for timing)


def kernel(x, layer0, core1, core2, last, ln_w, ln_b, trace=False,
           trace_kwargs=None):
    global _cached_nc, last_results
    from concourse.bass_utils import run_bass_kernel_spmd

    x = np.asarray(x, np.float32)
    consts = _prep_consts(layer0, core1, core2, last, ln_w, ln_b)

    if _cached_nc is None:
        _cached_nc = _build_program(BC // P)
    nc = _cached_nc

    xflat = np.ascontiguousarray(x.reshape(B, D * L))
    in_maps = []
    for ci in range(N_CORES):
        m = dict(consts)
        m["xs"] = xflat[ci * BC:(ci + 1) * BC]
        in_maps.append(m)

    kw = {}
    if trace:
        kw["trace"] = True
        kw.update(trace_kwargs or {})
    res = run_bass_kernel_spmd(nc, in_maps, list(range(N_CORES)), **kw)
    last_results = res
    full = np.concatenate(
        [np.asarray(res.results[ci]["out"]) for ci in range(N_CORES)], 0)
    return np.ascontiguousarray(full.astype(np.float32, copy=False))

